# revision 8
# baseline (speedup 1.0000x reference)
"""ConvLSTM stack (3 layers) + MLP head on 8 Trainium2 NeuronCores via Bass.

Sharding: data-parallel over batch B=64 -> 8 samples/core, weights
replicated (per sharding hint).  Per core the full forward runs on-chip:

- Layouts (bf16): activations channel-major [C partitions, s, y, x] with
  (33,34)-padded spatial frames so 2x2 'same' conv taps read zeros at the
  bottom/right edges.  Cell state c unpadded [C, s, 32, 32].
- Conv as matmul: contraction over input channels on partitions; each
  2x2 tap is one accumulating matmul into a PSUM gate tile [128, 512].
  L1's x-conv (Cin=1) uses per-(tap,sample) masked lhsT rows over the
  sample-major x tensor.
- Keras hard_sigmoid folded into weights: i/f/o gate rows pre-scaled by
  0.2, bias' = 0.2 b + 0.5, then clamp01 on-chip (tensor_scalar max/min).
- Gates drain PSUM->SBUF on ScalarE (Identity + per-partition bias),
  pointwise LSTM math on VectorE in bf16, tanh on ScalarE.
- Head: W4 [131072, 256] streamed from HBM in bf16 (pos-major blocks),
  accumulated into PSUM [8, 256] with h3 slices as stationary operand;
  W5/W6 via PE transposes; softmax on-chip; output [8, 2] fp32.

Dispatch: one jax.jit'd shard_map over the 8 axon devices, built once and
cached; weights are device-resident after the first call (axon link is
~70 ms RTT, ~10-50 MB/s, so per-call traffic is just x + [64,2] out).

Falls back to the jax-CPU reference path on any Bass-path failure.
"""
import sys
import numpy as np

B, T, C, H, W = 64, 8, 1, 32, 32
N_CORES = 8
BL = B // N_CORES          # 8 samples per core
PH, PW = 33, 34            # padded frame (even row stride for bf16 DVE modes)
NPOS = H * W               # 1024 interior positions
PFRAME = PH * PW           # 1122

_CACHE = {}


# ===================================================================== bass
def _build_nc(fix=True):
    import concourse.bass as bass
    import concourse.mybir as mybir
    from concourse import tile

    dt = mybir.dt
    AF = mybir.ActivationFunctionType
    OP = mybir.AluOpType
    TAPS = ((0, 0), (0, 1), (1, 0), (1, 1))

    nc = bass.Bass()

    # ---- DRAM parameters (per core) ----
    x_in = nc.declare_dram_parameter("x_in", [BL, T, H, W], dt.bfloat16, isOutput=False)
    wx1m = nc.declare_dram_parameter("wx1m", [BL, 4 * BL * 128], dt.bfloat16, isOutput=False)
    wh1 = nc.declare_dram_parameter("wh1", [32, 4 * 128], dt.bfloat16, isOutput=False)
    wx2 = nc.declare_dram_parameter("wx2", [32, 4 * 256], dt.bfloat16, isOutput=False)
    wh2 = nc.declare_dram_parameter("wh2", [64, 4 * 256], dt.bfloat16, isOutput=False)
    wx3 = nc.declare_dram_parameter("wx3", [64, 4 * 512], dt.bfloat16, isOutput=False)
    wh3 = nc.declare_dram_parameter("wh3", [128, 4 * 512], dt.bfloat16, isOutput=False)
    bias1 = nc.declare_dram_parameter("bias1", [128, 1], dt.float32, isOutput=False)
    bias2 = nc.declare_dram_parameter("bias2", [128, 2], dt.float32, isOutput=False)
    bias3 = nc.declare_dram_parameter("bias3", [128, 4], dt.float32, isOutput=False)
    w4r = nc.declare_dram_parameter("w4r", [NPOS, 128, 256], dt.bfloat16, isOutput=False)
    b4row = nc.declare_dram_parameter("b4row", [1, 256], dt.float32, isOutput=False)
    w5r = nc.declare_dram_parameter("w5r", [128, 2 * 1024], dt.bfloat16, isOutput=False)
    b5col = nc.declare_dram_parameter("b5col", [128, 8], dt.float32, isOutput=False)
    w6r = nc.declare_dram_parameter("w6r", [128, 8 * 2], dt.bfloat16, isOutput=False)
    b6row = nc.declare_dram_parameter("b6row", [1, 2], dt.float32, isOutput=False)
    ident8 = nc.declare_dram_parameter("ident8", [8, 8], dt.bfloat16, isOutput=False)
    ones18 = nc.declare_dram_parameter("ones18", [1, 8], dt.bfloat16, isOutput=False)
    y_out = nc.declare_dram_parameter("y_out", [BL, 2], dt.float32, isOutput=True)

    with tile.TileContext(nc) as tc:
        import contextlib
        ctx = contextlib.ExitStack()
        with ctx:
            P = ctx.enter_context(tc.tile_pool(name="state", bufs=1))
            GP = ctx.enter_context(tc.tile_pool(name="gates", bufs=1))
            WP = ctx.enter_context(tc.tile_pool(name="w4s", bufs=2))
            PS = ctx.enter_context(tc.tile_pool(name="psum", bufs=6, space="PSUM"))
            PS2 = ctx.enter_context(tc.tile_pool(name="psum2", bufs=1, space="PSUM"))

            # ---- persistent SBUF state ----
            xp = P.tile([BL, T, PH, PW], dt.bfloat16, tag="xp")
            h1p = P.tile([32, BL, PH, PW], dt.bfloat16, tag="h1p")
            h2p = P.tile([64, BL, PH, PW], dt.bfloat16, tag="h2p")
            h3p = P.tile([128, BL, PH, PW], dt.bfloat16, tag="h3p")
            c1 = P.tile([32, BL, H, W], dt.bfloat16, tag="c1")
            c2 = P.tile([64, BL, H, W], dt.bfloat16, tag="c2")
            c3 = P.tile([128, BL, H, W], dt.bfloat16, tag="c3")

            swx1 = P.tile([BL, 4 * BL * 128], dt.bfloat16, tag="swx1")
            swh1 = P.tile([32, 4 * 128], dt.bfloat16, tag="swh1")
            swx2 = P.tile([32, 4 * 256], dt.bfloat16, tag="swx2")
            swh2 = P.tile([64, 4 * 256], dt.bfloat16, tag="swh2")
            swx3 = P.tile([64, 4 * 512], dt.bfloat16, tag="swx3")
            swh3 = P.tile([128, 4 * 512], dt.bfloat16, tag="swh3")
            sb1 = P.tile([128, 1], dt.float32, tag="sb1")
            sb2 = P.tile([128, 2], dt.float32, tag="sb2")
            sb3 = P.tile([128, 4], dt.float32, tag="sb3")
            sw5 = P.tile([128, 2 * 1024], dt.bfloat16, tag="sw5")
            sb5 = P.tile([128, 8], dt.float32, tag="sb5")
            sw6 = P.tile([128, 8 * 2], dt.bfloat16, tag="sw6")
            sb4r = P.tile([1, 256], dt.float32, tag="sb4r")
            sb6r = P.tile([1, 2], dt.float32, tag="sb6r")
            sid8 = P.tile([8, 8], dt.bfloat16, tag="sid8")
            sone = P.tile([1, 8], dt.bfloat16, tag="sone")

            # zero-init padded frames and states
            for tz in (xp, h1p, h2p, h3p, c1, c2, c3):
                nc.vector.memset(tz[:], 0.0)

            # load inputs/weights
            for tt_ in range(T):
                nc.sync.dma_start(xp[:, tt_, 0:H, 0:W], x_in[:, tt_])
            for dst, src in ((swx1, wx1m), (swh1, wh1), (swx2, wx2), (swh2, wh2),
                             (swx3, wx3), (swh3, wh3), (sb1, bias1), (sb2, bias2),
                             (sb3, bias3), (sw5, w5r), (sb5, b5col), (sw6, w6r),
                             (sb4r, b4row), (sb6r, b6row), (sid8, ident8),
                             (sone, ones18)):
                nc.sync.dma_start(dst[:], src[:])

            # b4 row as bf16 for the ones-trick matmul (rhs dtype must match)
            sb4b = P.tile([1, 256], dt.bfloat16, tag="sb4b")
            nc.vector.tensor_copy(sb4b[:], sb4r[:])
            sb6b = P.tile([1, 2], dt.bfloat16, tag="sb6b")
            nc.vector.tensor_copy(sb6b[:], sb6r[:])

            def clamp01(ap):
                nc.vector.tensor_scalar(ap, ap, 0.0, 1.0, OP.max, OP.min)

            def pointwise(ga_i, ga_f, ga_g, ga_o, c_sl, h_dst):
                """LSTM cell update on one 2-sample chunk (free size 2048)."""
                clamp01(ga_i)
                clamp01(ga_f)
                clamp01(ga_o)
                nc.scalar.activation(ga_g, ga_g, AF.Tanh)
                nc.vector.tensor_tensor(ga_i, ga_i, ga_g, OP.mult)      # t1 = i*tanh(g)
                nc.vector.tensor_tensor(ga_f, ga_f, c_sl, OP.mult)      # t2 = f*c
                nc.vector.tensor_tensor(c_sl, ga_i, ga_f, OP.add)       # c = t1+t2
                nc.scalar.activation(ga_g, c_sl, AF.Tanh)               # tc = tanh(c)
                nc.vector.tensor_tensor(h_dst, ga_o, ga_g, OP.mult)     # h = o*tc

            # ---------------- ConvLSTM stack ----------------
            for t in range(T):
                for ch in range(BL // 2):     # chunks of 2 samples
                    s0 = 2 * ch
                    # ---- L1: gates [4g x 32c, 2048] ----
                    g1 = GP.tile([128, 2, 2, 512], dt.bfloat16, tag="g1")
                    for sl in range(2):
                        s = s0 + sl
                        for yh in range(2):
                            y0 = 16 * yh
                            ps = PS.tile([128, 512], dt.float32, tag="cps")
                            for ti, (kh, kw) in enumerate(TAPS):
                                nc.tensor.matmul(
                                    ps[:],
                                    swx1[:, (ti * BL + s) * 128:(ti * BL + s + 1) * 128],
                                    xp[:, t, y0 + kh:y0 + kh + 16, kw:kw + W],
                                    start=(ti == 0), stop=(t == 0 and ti == 3))
                            if t > 0:
                                for ti, (kh, kw) in enumerate(TAPS):
                                    nc.tensor.matmul(
                                        ps[:],
                                        swh1[:, ti * 128:(ti + 1) * 128],
                                        h1p[:, s, y0 + kh:y0 + kh + 16, kw:kw + W],
                                        start=False, stop=(ti == 3))
                            nc.scalar.activation(g1[:, sl, yh, :], ps[:],
                                                 AF.Identity, bias=sb1[:])
                    pointwise(g1[0:32].rearrange("p a b f -> p (a b f)"),
                              g1[32:64].rearrange("p a b f -> p (a b f)"),
                              g1[64:96].rearrange("p a b f -> p (a b f)"),
                              g1[96:128].rearrange("p a b f -> p (a b f)"),
                              c1[:, s0:s0 + 2].rearrange("p a b f -> p (a b f)"),
                              h1p[:, s0:s0 + 2, 0:H, 0:W])

                    # ---- L2: two gate tiles [i|f], [g|o] over 64c ----
                    g2a = GP.tile([128, 2, 2, 512], dt.bfloat16, tag="g2a")
                    g2b = GP.tile([128, 2, 2, 512], dt.bfloat16, tag="g2b")
                    for half, gdst, wcol in ((0, g2a, 0), (1, g2b, 128)):
                        for sl in range(2):
                            s = s0 + sl
                            for yh in range(2):
                                y0 = 16 * yh
                                ps = PS.tile([128, 512], dt.float32, tag="cps")
                                for ti, (kh, kw) in enumerate(TAPS):
                                    nc.tensor.matmul(
                                        ps[:],
                                        swx2[:, ti * 256 + wcol:ti * 256 + wcol + 128],
                                        h1p[:, s, y0 + kh:y0 + kh + 16, kw:kw + W],
                                        start=(ti == 0), stop=(t == 0 and ti == 3))
                                if t > 0:
                                    for ti, (kh, kw) in enumerate(TAPS):
                                        nc.tensor.matmul(
                                            ps[:],
                                            swh2[:, ti * 256 + wcol:ti * 256 + wcol + 128],
                                            h2p[:, s, y0 + kh:y0 + kh + 16, kw:kw + W],
                                            start=False, stop=(ti == 3))
                                nc.scalar.activation(gdst[:, sl, yh, :], ps[:],
                                                     AF.Identity,
                                                     bias=sb2[:, half:half + 1])
                    pointwise(g2a[0:64].rearrange("p a b f -> p (a b f)"),
                              g2a[64:128].rearrange("p a b f -> p (a b f)"),
                              g2b[0:64].rearrange("p a b f -> p (a b f)"),
                              g2b[64:128].rearrange("p a b f -> p (a b f)"),
                              c2[:, s0:s0 + 2].rearrange("p a b f -> p (a b f)"),
                              h2p[:, s0:s0 + 2, 0:H, 0:W])

                    # ---- L3: four gate tiles [128c, 2048] ----
                    g3_0 = GP.tile([128, 2, 2, 512], dt.bfloat16, tag="g3_0")
                    g3_1 = GP.tile([128, 2, 2, 512], dt.bfloat16, tag="g3_1")
                    g3_2 = GP.tile([128, 2, 2, 512], dt.bfloat16, tag="g3_2")
                    g3_3 = GP.tile([128, 2, 2, 512], dt.bfloat16, tag="g3_3")
                    g3 = [g3_0, g3_1, g3_2, g3_3]
                    for gi in range(4):
                        for sl in range(2):
                            s = s0 + sl
                            for yh in range(2):
                                y0 = 16 * yh
                                ps = PS.tile([128, 512], dt.float32, tag="cps")
                                for ti, (kh, kw) in enumerate(TAPS):
                                    nc.tensor.matmul(
                                        ps[:],
                                        swx3[:, ti * 512 + gi * 128:ti * 512 + gi * 128 + 128],
                                        h2p[:, s, y0 + kh:y0 + kh + 16, kw:kw + W],
                                        start=(ti == 0), stop=(t == 0 and ti == 3))
                                if t > 0:
                                    for ti, (kh, kw) in enumerate(TAPS):
                                        nc.tensor.matmul(
                                            ps[:],
                                            swh3[:, ti * 512 + gi * 128:ti * 512 + gi * 128 + 128],
                                            h3p[:, s, y0 + kh:y0 + kh + 16, kw:kw + W],
                                            start=False, stop=(ti == 3))
                                nc.scalar.activation(g3[gi][:, sl, yh, :], ps[:],
                                                     AF.Identity,
                                                     bias=sb3[:, gi:gi + 1])
                    pointwise(g3[0].rearrange("p a b f -> p (a b f)"),
                              g3[1].rearrange("p a b f -> p (a b f)"),
                              g3[2].rearrange("p a b f -> p (a b f)"),
                              g3[3].rearrange("p a b f -> p (a b f)"),
                              c3[:, s0:s0 + 2].rearrange("p a b f -> p (a b f)"),
                              h3p[:, s0:s0 + 2, 0:H, 0:W])

            # ---------------- dense head ----------------
            ps4 = PS2.tile([8, 256], dt.float32, tag="hps")
            for kt in range(128):           # 8 positions per W4 block
                wt = WP.tile([128, 8, 256], dt.bfloat16, tag="w4t")
                nc.sync.dma_start(
                    wt[:], w4r[kt * 8:(kt + 1) * 8, :, :].rearrange("i p j -> p i j"))
                for i in range(8):
                    pos = kt * 8 + i
                    yy, xx = pos // W, pos % W
                    nc.tensor.matmul(ps4[:], h3p[:, :, yy, xx], wt[:, i, :],
                                     start=(pos == 0), stop=False)
            nc.tensor.matmul(ps4[:], sone[:], sb4b[:], start=False, stop=True)
            a4 = P.tile([8, 256], dt.bfloat16, tag="a4")
            nc.scalar.activation(a4[:], ps4[:], AF.Relu)

            a4T = P.tile([128, 16], dt.bfloat16, tag="a4T")
            for j2 in range(2):
                pst = PS2.tile([128, 8], dt.bfloat16, tag="hpst")
                nc.tensor.transpose(pst[:], a4[:, j2 * 128:(j2 + 1) * 128], sid8[:])
                nc.scalar.activation(a4T[:, j2 * 8:(j2 + 1) * 8], pst[:], AF.Copy)

            a5T = P.tile([128, 64], dt.bfloat16, tag="a5T")
            for jt in range(8):
                ps5 = PS2.tile([128, 8], dt.float32, tag="hps")
                for kt2 in range(2):
                    nc.tensor.matmul(
                        ps5[:], sw5[:, kt2 * 1024 + jt * 128:kt2 * 1024 + (jt + 1) * 128],
                        a4T[:, kt2 * 8:(kt2 + 1) * 8],
                        start=(kt2 == 0), stop=(kt2 == 1))
                nc.scalar.activation(a5T[:, jt * 8:(jt + 1) * 8], ps5[:], AF.Relu,
                                     bias=sb5[:, jt:jt + 1])

            ps6 = PS2.tile([8, 2], dt.float32, tag="hps")
            for jt in range(8):
                nc.tensor.matmul(ps6[:], a5T[:, jt * 8:(jt + 1) * 8],
                                 sw6[:, jt * 2:(jt + 1) * 2],
                                 start=(jt == 0), stop=False)
            nc.tensor.matmul(ps6[:], sone[:], sb6b[:], start=False, stop=True)

            esb = P.tile([8, 2], dt.float32, tag="esb")
            nc.scalar.activation(esb[:], ps6[:], AF.Exp)
            ssum = P.tile([8, 1], dt.float32, tag="ssum")
            nc.vector.tensor_reduce(ssum[:], esb[:], mybir.AxisListType.X, OP.add)
            rinv = P.tile([8, 1], dt.float32, tag="rinv")
            nc.vector.reciprocal(rinv[:], ssum[:])
            osb = P.tile([8, 2], dt.float32, tag="osb")
            nc.vector.tensor_scalar(osb[:], esb[:], rinv[:], None, OP.mult)
            nc.sync.dma_start(y_out[:], osb[:])

    if fix:
        _fix_sync_waits(nc)
    return nc


def _fix_sync_waits(nc, max_waits=1):
    """This walrus build rejects >1 sync-wait per instruction; hoist the
    excess onto preceding same-engine NoOps (equivalent blocking)."""
    import concourse.mybir as mybir
    k = 0
    for f in nc.m.functions:
        for bb in f.blocks:
            new_insts = []
            for i in bb.instructions:
                si = i.sync_info
                if si is not None and si.on_wait and len(si.on_wait) > max_waits:
                    waits = list(si.on_wait)
                    for w in waits[:-max_waits]:
                        k += 1
                        new_insts.append(mybir.InstNoOp(
                            name=f"I-waitfix-{k}", engine=i.engine, ins=[], outs=[],
                            sync_info=mybir.SyncInfo(on_wait=[w], on_update=[])))
                    si.on_wait = waits[-max_waits:]
                new_insts.append(i)
            bb.instructions[:] = new_insts


# ============================================================ host weights
def _bf16(a):
    import ml_dtypes
    return np.asarray(a, np.float32).astype(ml_dtypes.bfloat16)


def _prep_weights(inputs):
    """Host-side weight transforms (gate scaling folded, matmul layouts)."""
    g = lambda k: np.asarray(inputs[k], np.float32)

    def gate_scale(w, b, F):
        # Keras order i,f,g,o along axis0 blocks of F; scale i/f/o by 0.2,
        # bias' = 0.2 b + 0.5 for those gates.
        w = w.copy(); b = b.copy()
        for gi in (0, 1, 3):
            w[gi * F:(gi + 1) * F] *= 0.2
        b2 = b.copy()
        for gi in (0, 1, 3):
            b2[gi * F:(gi + 1) * F] = 0.2 * b[gi * F:(gi + 1) * F] + 0.5
        return w, b2

    out = {}
    # ---- L1 ----
    Wx1, b1 = gate_scale(g("Wx1"), g("b1"), 32)       # [128,1,2,2]
    Wh1, _ = gate_scale(g("Wh1"), g("b1"), 32)
    # masked per-(tap, sample) lhsT rows: [BL, 4*BL*128]
    wx1m = np.zeros((BL, 4 * BL * 128), np.float32)
    for ti in range(4):
        kh, kw = ti // 2, ti % 2
        for s in range(BL):
            wx1m[s, (ti * BL + s) * 128:(ti * BL + s + 1) * 128] = Wx1[:, 0, kh, kw]
    out["wx1m"] = _bf16(wx1m)
    wh1 = np.zeros((32, 4 * 128), np.float32)
    for ti in range(4):
        kh, kw = ti // 2, ti % 2
        wh1[:, ti * 128:(ti + 1) * 128] = Wh1[:, :, kh, kw].T
    out["wh1"] = _bf16(wh1)
    # bias1 packed [4g*32c]
    out["bias1"] = b1.reshape(128, 1).astype(np.float32)

    # ---- L2 ----
    Wx2, b2 = gate_scale(g("Wx2"), g("b2"), 64)       # [256,32,2,2]
    Wh2, _ = gate_scale(g("Wh2"), g("b2"), 64)
    wx2 = np.zeros((32, 4 * 256), np.float32)
    wh2 = np.zeros((64, 4 * 256), np.float32)
    for ti in range(4):
        kh, kw = ti // 2, ti % 2
        wx2[:, ti * 256:(ti + 1) * 256] = Wx2[:, :, kh, kw].T
        wh2[:, ti * 256:(ti + 1) * 256] = Wh2[:, :, kh, kw].T
    out["wx2"] = _bf16(wx2)
    out["wh2"] = _bf16(wh2)
    out["bias2"] = b2.reshape(2, 128).T.copy().astype(np.float32)  # cols: [i|f],[g|o]

    # ---- L3 ----
    Wx3, b3 = gate_scale(g("Wx3"), g("b3"), 128)      # [512,64,2,2]
    Wh3, _ = gate_scale(g("Wh3"), g("b3"), 128)
    wx3 = np.zeros((64, 4 * 512), np.float32)
    wh3 = np.zeros((128, 4 * 512), np.float32)
    for ti in range(4):
        kh, kw = ti // 2, ti % 2
        wx3[:, ti * 512:(ti + 1) * 512] = Wx3[:, :, kh, kw].T
        wh3[:, ti * 512:(ti + 1) * 512] = Wh3[:, :, kh, kw].T
    out["wx3"] = _bf16(wx3)
    out["wh3"] = _bf16(wh3)
    out["bias3"] = b3.reshape(4, 128).T.copy().astype(np.float32)

    # ---- head ----
    W4 = g("W4")                                       # [128*1024, 256]
    out["w4r"] = _bf16(W4.reshape(128, NPOS, 256).transpose(1, 0, 2).copy())
    out["b4row"] = g("b4").reshape(1, 256).astype(np.float32)
    W5 = g("W5")                                       # [256, 1024]
    out["w5r"] = _bf16(W5.reshape(2, 128, 1024).transpose(1, 0, 2).reshape(128, 2048).copy())
    out["b5col"] = g("b5").reshape(8, 128).T.copy().astype(np.float32)
    W6 = g("W6")                                       # [1024, 2]
    out["w6r"] = _bf16(W6.reshape(8, 128, 2).transpose(1, 0, 2).reshape(128, 16).copy())
    out["b6row"] = g("b6").reshape(1, 2).astype(np.float32)
    out["ident8"] = _bf16(np.eye(8, dtype=np.float32))
    out["ones18"] = _bf16(np.ones((1, 8), np.float32))
    return out


# ============================================================== dispatcher
def _get_runner():
    if "runner" in _CACHE:
        return _CACHE["runner"]
    import jax
    import concourse.mybir as mybir
    from jax.sharding import Mesh, PartitionSpec, NamedSharding
    from jax.experimental.shard_map import shard_map
    from concourse.bass2jax import _bass_exec_p, install_neuronx_cc_hook, \
        partition_id_tensor

    nc = _build_nc()
    install_neuronx_cc_hook()
    partition_name = nc.partition_id_tensor.name if nc.partition_id_tensor else None
    in_names, out_names, out_avals = [], [], []
    for alloc in nc.m.functions[0].allocations:
        if not isinstance(alloc, mybir.MemoryLocationSet):
            continue
        name = alloc.memorylocations[0].name
        if alloc.kind == "ExternalInput":
            if name != partition_name:
                in_names.append(name)
        elif alloc.kind == "ExternalOutput":
            out_names.append(name)
            out_avals.append(jax.core.ShapedArray(
                tuple(alloc.tensor_shape), mybir.dt.np(alloc.dtype)))
    n_params = len(in_names)
    n_outs = len(out_avals)
    all_in = list(in_names) + list(out_names)
    if partition_name is not None:
        all_in.append(partition_name)

    def _body(*args):
        operands = list(args)
        if partition_name is not None:
            operands.append(partition_id_tensor())
        return tuple(_bass_exec_p.bind(
            *operands, out_avals=tuple(out_avals), in_names=tuple(all_in),
            out_names=tuple(out_names), lowering_input_output_aliases=(),
            sim_require_finite=False, sim_require_nnan=False, nc=nc))

    devices = jax.devices()[:N_CORES]
    mesh = Mesh(np.asarray(devices), ("core",))
    fn = jax.jit(
        shard_map(_body, mesh=mesh,
                  in_specs=(PartitionSpec("core"),) * (n_params + n_outs),
                  out_specs=(PartitionSpec("core"),) * n_outs,
                  check_rep=False),
        keep_unused=True)
    sharding = NamedSharding(mesh, PartitionSpec("core"))
    runner = dict(fn=fn, in_names=in_names, out_names=out_names,
                  out_avals=out_avals, sharding=sharding, n_outs=n_outs)
    _CACHE["runner"] = runner
    return runner


def _bass_forward(inputs):
    import jax
    r = _get_runner()

    wkey = id(inputs.get("W4", None))
    if _CACHE.get("wkey") != wkey:
        prep = _prep_weights(inputs)
        dev = {}
        for name, arr in prep.items():
            # replicate per core, concat on axis0 for shard_map
            rep = np.concatenate([arr] * N_CORES, axis=0)
            dev[name] = jax.device_put(rep, r["sharding"])
        # zero output donors (device-resident, not donated)
        for i, av in enumerate(r["out_avals"]):
            z = np.zeros((N_CORES * av.shape[0],) + tuple(av.shape[1:]),
                         av.dtype)
            dev[f"__out{i}"] = jax.device_put(z, r["sharding"])
        _CACHE["wdev"] = dev
        _CACHE["wkey"] = wkey
    dev = _CACHE["wdev"]

    # x: [B, T, 1, H, W] -> per-core [BL, T, H, W] bf16, concat on axis0
    x = np.asarray(inputs["x"], np.float32).reshape(B, T, H, W)
    xb = _bf16(x)
    x_dev = jax.device_put(xb, r["sharding"])

    args = []
    for n in r["in_names"]:
        args.append(x_dev if n == "x_in" else dev[n])
    args += [dev[f"__out{i}"] for i in range(r["n_outs"])]
    outs = r["fn"](*args)
    y = np.asarray(outs[0]).reshape(B, 2).astype(np.float32)
    return y


# ================================================================ fallback
def _jax_cpu_forward(inputs):
    import jax
    import jax.numpy as jnp
    from jax import lax

    cpu = jax.devices("cpu")[0]

    def conv(x, w):
        return lax.conv_general_dilated(
            x, w, (1, 1), [(0, 1), (0, 1)],
            dimension_numbers=("NCHW", "OIHW", "NCHW"))

    def hsig(x):
        return jnp.clip(0.2 * x + 0.5, 0.0, 1.0)

    def convlstm(xs, Wx, Wh, b, return_seq):
        F = b.shape[0] // 4
        b_, _, Hh, Ww = xs[0].shape
        h = jnp.zeros((b_, F, Hh, Ww), xs[0].dtype)
        c = jnp.zeros((b_, F, Hh, Ww), xs[0].dtype)
        outs = []
        for t in range(T):
            z = conv(xs[t], Wx) + conv(h, Wh) + b[None, :, None, None]
            i, f, g, o = jnp.split(z, 4, axis=1)
            i = hsig(i); f = hsig(f); o = hsig(o)
            c = f * c + i * jnp.tanh(g)
            h = o * jnp.tanh(c)
            outs.append(h)
        return outs if return_seq else h

    def model(x, Wx1, Wh1, b1, Wx2, Wh2, b2, Wx3, Wh3, b3, W4, b4, W5, b5, W6, b6):
        xs = [x[:, t] for t in range(T)]
        h1 = convlstm(xs, Wx1, Wh1, b1, True)
        h2 = convlstm(h1, Wx2, Wh2, b2, True)
        h3 = convlstm(h2, Wx3, Wh3, b3, False)
        f = h3.reshape(h3.shape[0], -1)
        a = jax.nn.relu(f @ W4 + b4)
        a = jax.nn.relu(a @ W5 + b5)
        return jax.nn.softmax(a @ W6 + b6, axis=-1)

    if "cpu_jit" not in _CACHE:
        _CACHE["cpu_jit"] = jax.jit(model, device=cpu)
    fn = _CACHE["cpu_jit"]
    xs = np.ascontiguousarray(inputs["x"].reshape(B, T, C, H, W), dtype=np.float32)
    args = [np.asarray(inputs[k], np.float32) for k in
            ("Wx1", "Wh1", "b1", "Wx2", "Wh2", "b2", "Wx3", "Wh3", "b3",
             "W4", "b4", "W5", "b5", "W6", "b6")]
    return np.asarray(fn(xs, *args), np.float32).reshape(B, 2)


def kernel(**inputs) -> np.ndarray:
    if _CACHE.get("bass_broken"):
        return _jax_cpu_forward(inputs)
    try:
        return _bass_forward(inputs)
    except Exception as ex:
        print(f"kernel: bass path failed ({type(ex).__name__}: {ex}); "
              f"falling back to jax-CPU", file=sys.stderr)
        _CACHE["bass_broken"] = True
        return _jax_cpu_forward(inputs)
